# revision 79
# baseline (speedup 1.0000x reference)
"""Causal GQA attention with RoPE for Trainium2, sharded over 8 NeuronCores.

Problem: x[4,1024,2048] @ wq/wk/wv -> RoPE -> causal GQA attention -> @ wo.
H=32 q-heads, KVH=8 kv-heads (GQA rep 4), D=64.

Sharding: core = 2*b + g  (b = batch 0..3, g = head-group 0..1).
Each core handles one batch and 16 q-heads / 4 kv-heads, computing a partial
output projection; the host sums the two head-group partials per batch.

v2 layout/engine choices:
  - all matmul operands are bf16 (psum accumulation stays fp32): full-rate
    PE at any moving width (fp32r pays 4x below 256-wide), half-size DMAs,
    2x DVE elementwise on bf16 tensors.
  - K/V projections run contraction-outer in one sweep so the PE consumes
    each xT chunk group the moment its DMA lands; the V accumulators pack
    two key-chunk groups per PSUM bank (one spanning accumulation group
    per bank since start=True claims the whole 2KB zero region). The last
    xT group's K chunks run before V's so the K banks close early.
  - two KV PSUM pools (K banks / V banks) so the attention-era pools that
    alias the K banks don't wait for the V-side drain; all per-bank
    handoffs are WAR dependencies.
  - initial DMAs are split fine-grained and ordered so the first K matmul
    can issue ~3us in; weight tensors are fetched in 256-column pairs to
    keep DMA descriptors >= 512B (bf16 halves the row size).
  - rope: one PSUM->SBUF bf16 stage copy (ACT for K ropes where ACT is
    idle, DVE for Q ropes where ACT is exp-saturated), then 2-byte
    SBUF-only DVE ops in 2x/4x perf modes. TensorTensor with both inputs
    in SBUF must be partition-aligned, so the 32-row head-dim block swap
    rides on single-input copies of u = q * sin_preswapped (the host
    stores sin already swapped).
  - engine balance: ACT does exp + V/osb PSUM copies in phases where it is
    otherwise idle, DVE does rope/mask/normalize, GpSimd does the kdup
    head duplication, rowsum reciprocal broadcast and memsets.
  - softmax without max-subtraction; row-sum over keys via a ones-column
    appended to V (matmul cost is moving-width only, so the 65th output
    partition is free).
  - causal masking: fully-masked key blocks skipped; diagonal blocks
    compute the reachable column range plus one 128-wide triangle multiply.
"""

import os

import ml_dtypes
import numpy as np

import concourse.bacc as bacc
import concourse.bass as bass
import concourse.mybir as mybir
import concourse.tile as tile
from concourse.bass_utils import run_bass_kernel_spmd

B, S, DIM = 4, 1024, 2048
H, KVH, D = 32, 8, 64
HL = H // 2        # 16 q heads per core
KVL = KVH // 2     # 4 kv heads per core
QCOLS = HL * D     # 1024
KCOLS = KVL * D    # 256
NB = 512           # matmul moving-dim block (one PSUM bank of fp32)
P = 128

F32 = mybir.dt.float32
BF16 = mybir.dt.bfloat16


def build_program():
    nc = bacc.Bacc()

    xT = nc.dram_tensor("xT", [DIM, S], BF16, kind="ExternalInput")
    wq = nc.dram_tensor("wq", [DIM, QCOLS], BF16, kind="ExternalInput")
    wk = nc.dram_tensor("wk", [DIM, KCOLS], BF16, kind="ExternalInput")
    wv = nc.dram_tensor("wv", [DIM, KCOLS], BF16, kind="ExternalInput")
    wo = nc.dram_tensor("wo", [QCOLS, DIM], BF16, kind="ExternalInput")
    cosP = nc.dram_tensor("cosP", [P, S], BF16, kind="ExternalInput")
    sinP = nc.dram_tensor("sinP", [P, S], BF16, kind="ExternalInput")
    maskb = nc.dram_tensor("maskb", [P, P], BF16, kind="ExternalInput")
    outA = nc.dram_tensor("outA", [DIM, S], BF16, kind="ExternalOutput")

    KC = DIM // P   # 16 contraction chunks
    Exp = mybir.ActivationFunctionType.Exp

    with tile.TileContext(nc) as tc:
        from contextlib import ExitStack
        es = ExitStack()
        with es:
            const = es.enter_context(tc.tile_pool(name="const", bufs=1))
            kdupp = es.enter_context(tc.tile_pool(name="kdupp", bufs=1))
            vaugp = es.enter_context(tc.tile_pool(name="vaugp", bufs=1))
            aotp = es.enter_context(tc.tile_pool(name="aotp", bufs=1))
            qrtp = es.enter_context(tc.tile_pool(name="qrtp", bufs=2))
            spool = es.enter_context(tc.tile_pool(name="spool", bufs=2))
            epool = es.enter_context(tc.tile_pool(name="epool", bufs=4))
            rpool = es.enter_context(tc.tile_pool(name="rpool", bufs=2))

            # ---- constants ----
            cost = const.tile([P, S], BF16, name="cost")
            sint = const.tile([P, S], BF16, name="sint")
            maskt = const.tile([P, P], BF16, name="maskt")

            # persistent activation tiles
            kdup = [kdupp.tile([P, S], BF16, name=f"kdup{i}") for i in range(KVL)]
            # per key-chunk: all 4 kv heads' V columns + a ones column each
            # ([128, 4*(D+1)]); one strided copy fills all 4 heads at once
            vaug = [vaugp.tile([P, KVL * (D + 1)], BF16, name=f"vaug{ic}")
                    for ic in range(S // P)]
            aot = [aotp.tile([P, S], BF16, name=f"aot{j}") for j in range(8)]

            def rope(ps, ib, dest_ap, stage_eng="act"):
                """psum [128, NB] fp32 -> roped into dest_ap (bf16).

                One copy stages the PSUM block into SBUF as bf16; all
                remaining ops are then 2-byte SBUF-only DVE tensor-tensor
                ops which run in the DVE's 2x/4x perf modes. The stage
                copy goes on ACT for the K ropes (ACT is idle then) and on
                DVE for the Q ropes (ACT is exp-saturated in attention)."""
                sl = slice(ib * NB, (ib + 1) * NB)
                sb = spool.tile([P, NB], BF16, tag="ropesb", bufs=4)
                if stage_eng == "act":
                    nc.scalar.copy(sb[:], ps[:])
                else:
                    nc.vector.tensor_copy(sb[:], ps[:])
                # u = sb * swap(sin); then sw = swap(u) = swap(sb) * sin.
                # (TensorTensor with both inputs in SBUF must be partition-
                # aligned, so the swap rides on single-input copies instead;
                # sint holds the PRE-swapped sin values from the host.)
                u = spool.tile([P, NB], BF16, tag="ropeu", bufs=4)
                nc.vector.tensor_mul(u[:], sb[:], sint[:, sl])
                sw = spool.tile([P, NB], BF16, tag="swapt", bufs=4)
                nc.vector.tensor_copy(sw[0:32, :], u[32:64, :])
                nc.vector.tensor_copy(sw[32:64, :], u[0:32, :])
                nc.vector.tensor_copy(sw[64:96, :], u[96:128, :])
                nc.vector.tensor_copy(sw[96:128, :], u[64:96, :])
                st = spool.tile([P, NB], BF16, tag="straight", bufs=4)
                nc.vector.tensor_mul(st[:], sb[:], cost[:, sl])
                nc.vector.tensor_add(dest_ap, st[:], sw[:])

            inner = ExitStack()
            with inner:
                xtp = inner.enter_context(tc.tile_pool(name="xtp", bufs=1))
                wkvp = inner.enter_context(tc.tile_pool(name="wkvp", bufs=1))
                wstp = inner.enter_context(tc.tile_pool(name="wstp", bufs=2))
                kv_es_a = ExitStack()
                kv_es_b = ExitStack()
                psum_kva = kv_es_a.enter_context(
                    tc.tile_pool(name="psum_kva", bufs=1, space="PSUM"))
                psum_kvb = kv_es_b.enter_context(
                    tc.tile_pool(name="psum_kvb", bufs=1, space="PSUM"))

                # ---- input DMAs, ordered + sized so compute starts ASAP ----
                wkall = wkvp.tile([P, KC * KCOLS], BF16, name="wkall")
                wvall = wkvp.tile([P, KC * KCOLS], BF16, name="wvall")
                xtg = [xtp.tile([P, 4 * S], BF16, name=f"xtg{g}")
                       for g in range(4)]
                xt = [xtg[c // 4][:, (c % 4) * S:(c % 4 + 1) * S]
                      for c in range(KC)]

                def load_wsplit(dst, src, g):
                    nc.sync.dma_start(
                        dst[:, g * 4 * KCOLS:(g + 1) * 4 * KCOLS].rearrange(
                            "p (c e) -> p c e", c=4),
                        src[g * 4 * P:(g + 1) * 4 * P, :].rearrange(
                            "(c p) e -> p c e", p=P))

                # the very first K matmul needs only wk's c=0 rows and the
                # first half of x chunk 0 — land those first, tiny
                nc.sync.dma_start(
                    wkall[:, 0:KCOLS], wk[0:P, :])
                nc.sync.dma_start(
                    xtg[0][:, 0:NB], xT[0:P, 0:NB])
                nc.sync.dma_start(
                    xtg[0][:, NB:S], xT[0:P, NB:S])
                nc.sync.dma_start(
                    wkall[:, KCOLS:4 * KCOLS].rearrange(
                        "p (c e) -> p c e", c=3),
                    wk[P:4 * P, :].rearrange("(c p) e -> p c e", p=P))
                # rest of the first x group as singles for fast arrival
                for cc in range(1, 4):
                    nc.sync.dma_start(
                        xtg[0][:, cc * S:(cc + 1) * S],
                        xT[cc * P:(cc + 1) * P, :])
                load_wsplit(wvall, wv, 0)
                for g in range(1, 4):
                    load_wsplit(wkall, wk, g)
                    nc.sync.dma_start(
                        xtg[g][:].rearrange("p (c e) -> p c e", c=4),
                        xT[g * 4 * P:(g + 1) * 4 * P, :].rearrange(
                            "(c p) e -> p c e", p=P))
                    load_wsplit(wvall, wv, g)
                nc.sync.dma_start(cost[:], cosP[:])
                nc.sync.dma_start(sint[:], sinP[:])
                nc.sync.dma_start(maskt[:], maskb[:])
                # (wq pair 0 is issued here, right behind the KV inputs, so
                # Q-projection can start the moment the KV sweep drains)

                # ones column of the augmented-V tiles (free rowsum trick)
                for ic in range(S // P):
                    nc.gpsimd.memset(
                        vaug[ic][:].rearrange(
                            "p (k e) -> p k e", e=D + 1)[:, :, D:D + 1], 1.0)

                # ---- K + V projections, contraction-outer ----
                # kps0..3 take PSUM banks 0-3; the V tiles pack two ic
                # accumulation groups per bank (banks 4-7), so the whole
                # K+V projection runs as one contraction-outer sweep.
                # Two separate pools so the attention-era pools that alias
                # the K banks don't have to wait for the V-side drain.
                kps = [psum_kva.tile([P, NB], F32, name=f"kps{i}")
                       for i in range(4)]  # jk*2 + ib
                vpp = [psum_kvb.tile([P, 2 * KCOLS], F32, name=f"vpp{i}")
                       for i in range(4)]  # ic pairs (2i, 2i+1)

                def emit_k_c(c):
                    for jk in range(2):
                        wsl = wkall[:, c * KCOLS + jk * P:
                                    c * KCOLS + (jk + 1) * P]
                        for ib in range(2):
                            nc.tensor.matmul(
                                kps[2 * jk + ib][:], wsl,
                                xt[c][:, ib * NB:(ib + 1) * NB],
                                start=(c == 0), stop=(c == KC - 1))

                def emit_v_c(c):
                    # two ic accumulation chains share each PSUM bank; a
                    # start=True claims the whole 2KB zero region, so only
                    # the bank's first matmul starts and its last one stops
                    # (per-element has_written handles the second chain's
                    # first write correctly)
                    for ic in range(8):
                        vt = vpp[ic // 2][:, (ic % 2) * KCOLS:
                                          (ic % 2 + 1) * KCOLS]
                        nc.tensor.matmul(
                            vt, xt[c][:, ic * P:(ic + 1) * P],
                            wvall[:, c * KCOLS:(c + 1) * KCOLS],
                            start=(c == 0 and ic % 2 == 0),
                            stop=(c == KC - 1 and ic % 2 == 1),
                            skip_group_check=True)

                # c = 0..11 arrive while the PE is DMA-bound: interleave K
                # and V per chunk. The last group (c = 12..15) lands with
                # everything already queued, so run K's chunks first — the
                # K banks close ~4us earlier and the rope chain + the
                # Q-projection (whose pool aliases the K banks) start early.
                for c in range(12):
                    emit_k_c(c)
                    emit_v_c(c)
                for c in range(12, KC):
                    emit_k_c(c)
                for c in range(12, KC):
                    emit_v_c(c)

                # K rope + head-duplication (scores stationary needs the kv
                # head present on both 64-partition halves); dups ride on
                # the otherwise-idle GpSimd so the DVE only does the ropes.
                # V copies are one strided ACT op per key chunk, interleaved
                # between the rope stage copies so the PSUM banks drain fast.
                def emit_krope(jk, ib):
                    kr = spool.tile([P, NB], BF16, tag="ropek", bufs=4)
                    with tc.high_priority():
                        rope(kps[2 * jk + ib], ib, kr[:])
                    sl = slice(ib * NB, (ib + 1) * NB)
                    for half in range(2):
                        src = kr[64 * half:64 * half + 64, :]
                        nc.gpsimd.tensor_copy(
                            kdup[2 * jk + half][0:64, sl], src)
                        nc.gpsimd.tensor_copy(
                            kdup[2 * jk + half][64:128, sl], src)

                def emit_vcopy(ic):
                    vt = vpp[ic // 2][:, (ic % 2) * KCOLS:
                                      (ic % 2 + 1) * KCOLS]
                    nc.scalar.copy(
                        vaug[ic][:].rearrange(
                            "p (k e) -> p k e", e=D + 1)[:, :, 0:D],
                        vt.rearrange("p (k e) -> p k e", e=D))

                # all four rope stage-copies first: they are the only readers
                # of the K banks, and the Q-projection pool aliases those
                emit_krope(0, 0)
                emit_krope(0, 1)
                emit_krope(1, 0)
                emit_krope(1, 1)
                for ic in range(8):
                    emit_vcopy(ic)
                kv_es_b.close()  # (LIFO close; releases are tracked per-zone)
                kv_es_a.close()

                # ---- Q projection + attention, interleaved ----
                # pool creation order = PSUM bank assignment order; mm
                # aliases the earliest-freed KV banks (kps0/1)
                psum = inner.enter_context(
                    tc.tile_pool(name="psum", bufs=3, space="PSUM"))
                psum_sc = inner.enter_context(
                    tc.tile_pool(name="psum_sc", bufs=3, space="PSUM"))
                psum_oa = inner.enter_context(
                    tc.tile_pool(name="psum_oa", bufs=2, space="PSUM"))

                def load_wq_pair(pair):
                    wqg = wstp.tile([P, KC * 2 * P], BF16, tag="wqpair")
                    nc.sync.dma_start(
                        wqg[:].rearrange("p (c e) -> p c e", c=KC),
                        wq[:, pair * 2 * P:(pair + 1) * 2 * P].rearrange(
                            "(c p) e -> p c e", p=P))
                    return wqg

                def emit_qk_ib(wqg, jq, qr, ib):
                    ps = psum.tile([P, NB], F32, tag="mm")
                    off = (jq % 2) * P
                    for c in range(KC):
                        nc.tensor.matmul(
                            ps[:], wqg[:, c * 2 * P + off:c * 2 * P + off + P],
                            xt[c][:, ib * NB:(ib + 1) * NB],
                            start=(c == 0), stop=(c == KC - 1))
                    rope(ps, ib, qr[:, ib * NB:(ib + 1) * NB],
                         stage_eng="dve")

                def emit_attention(jq, qr, ponly=None):
                    """Attention for the two heads in q-chunk jq."""
                    kvh = jq // 2
                    for p in ((0, 1) if ponly is None else (ponly,)):
                        hsl = slice(64 * p, 64 * p + 64)
                        for qb in range(S // NB):   # query 512-blocks
                            nkj = 4 * (qb + 1)      # causal key chunks
                            oa = psum_oa.tile([D + 1, NB], F32, tag="oa")
                            for kj in range(nkj):
                                # diagonal blocks: only columns >= 128*c can
                                # be unmasked, so compute the narrowed range
                                c = kj - (nkj - 4)
                                off = P * c if c > 0 else 0
                                w = NB - off
                                sps = psum_sc.tile([P, NB], F32, tag="sc")
                                nc.tensor.matmul(
                                    sps[:, 0:w],
                                    kdup[kvh][hsl, kj * P:(kj + 1) * P],
                                    qr[hsl, qb * NB + off:(qb + 1) * NB],
                                    start=True, stop=True)
                                E = epool.tile([P, NB], BF16, tag="E")
                                nc.scalar.activation(E[:, 0:w], sps[:, 0:w],
                                                     Exp)
                                if c >= 0:
                                    # triangular mask on the leading 128 cols
                                    nc.vector.tensor_mul(
                                        E[:, 0:P], E[:, 0:P], maskt[:])
                                nc.tensor.matmul(
                                    oa[:, off:NB],
                                    vaug[kj][:, kvh * (D + 1):
                                             (kvh + 1) * (D + 1)],
                                    E[:, 0:w],
                                    start=(kj == 0), stop=(kj == nkj - 1))
                            rec = rpool.tile([1, NB], F32, tag="rec")
                            nc.vector.reciprocal(rec[:], oa[D:D + 1, :])
                            bcs = rpool.tile([D, NB], F32, tag="bcs")
                            # broadcast 1/rowsum along partitions (idle GpSimd)
                            nc.gpsimd.partition_broadcast(bcs[:], rec[:])
                            qsl = slice(qb * NB, (qb + 1) * NB)
                            # cross-partition-base output for the odd head
                            nc.vector.tensor_mul(
                                aot[jq][64 * p:64 * p + D, qsl],
                                oa[0:D, :], bcs[:])

                # wo weights + A-half partial tiles, hoisted so the split
                # wo groups below can interleave with the final attention
                wop = inner.enter_context(tc.tile_pool(name="wop", bufs=8))
                outp = inner.enter_context(tc.tile_pool(name="outp", bufs=3))
                woap = inner.enter_context(tc.tile_pool(name="woap", bufs=1))

                def load_wo_pair(pair):
                    wog = wop.tile([P, 8 * 2 * P], BF16, tag="wot")
                    nc.sync.dma_start(
                        wog[:].rearrange("p (c e) -> p c e", c=8),
                        wo[:, pair * 2 * P:(pair + 1) * 2 * P].rearrange(
                            "(c p) e -> p c e", p=P))
                    return wog

                # the first 3 output chunks are computed in two half-
                # contractions: the hd0-3 half doesn't depend on the last
                # attention chunks, so it backfills the exp-bound PE bubbles
                # of the final attention; the halves are summed with one
                # DVE add when the hd4-7 half lands
                SPLIT_N = 6
                osba = {(n, ib): woap.tile([P, NB], BF16,
                                           name=f"osba{n}_{ib}")
                        for n in range(SPLIT_N) for ib in range(2)}

                def emit_woa(wog, n, ib):
                    off = (n % 2) * P
                    fps = psum.tile([P, NB], F32, tag="mm")
                    for hd in range(4):
                        nc.tensor.matmul(
                            fps[:],
                            wog[:, hd * 2 * P + off:hd * 2 * P + off + P],
                            aot[hd][:, ib * NB:(ib + 1) * NB],
                            start=(hd == 0), stop=(hd == 3))
                    nc.vector.tensor_copy(osba[(n, ib)][:], fps[:])

                wqgs = {0: load_wq_pair(0)}
                prev = None
                for jq in range(QCOLS // P):  # 8 q chunks
                    pair = jq // 2
                    if jq % 2 == 0 and pair + 1 < 4:
                        wqgs[pair + 1] = load_wq_pair(pair + 1)
                    wqg = wqgs[pair]
                    qr = qrtp.tile([P, S], BF16, tag="qr")
                    emit_qk_ib(wqg, jq, qr, 0)
                    if prev is not None:
                        emit_attention(prev[0], prev[1], ponly=0)
                    emit_qk_ib(wqg, jq, qr, 1)
                    if prev is not None:
                        emit_attention(prev[0], prev[1], ponly=1)
                    prev = (jq, qr)
                wogs = {p: load_wo_pair(p) for p in range(3)}
                emit_attention(prev[0], prev[1], ponly=0)
                for n in range(2):
                    emit_woa(wogs[n // 2], n, 0)
                    emit_woa(wogs[n // 2], n, 1)
                emit_attention(prev[0], prev[1], ponly=1)
                for n in range(2, SPLIT_N):
                    emit_woa(wogs[n // 2], n, 0)
                    emit_woa(wogs[n // 2], n, 1)

                # ---- output projection (wo) ----
                # split (4-matmul B-half) and full (8-matmul) chunks are
                # interleaved so the full groups pad the B-halves' psum
                # bank drains; all weight pairs were preloaded while the
                # DMA engines idled during attention
                for p in range(3, 8):
                    wogs[p] = load_wo_pair(p)
                order = [0, 6, 1, 7, 2, 8, 3, 9, 4, 10, 5,
                         11, 12, 13, 14, 15]
                for n in order:
                    pair = n // 2
                    wog = wogs[pair]
                    off = (n % 2) * P
                    osb = outp.tile([P, S], BF16, tag="osb")
                    for ib in range(S // NB):
                        fps = psum.tile([P, NB], F32, tag="mm")
                        hd0 = 4 if n < SPLIT_N else 0
                        for hd in range(hd0, 8):
                            nc.tensor.matmul(
                                fps[:],
                                wog[:, hd * 2 * P + off:
                                    hd * 2 * P + off + P],
                                aot[hd][:, ib * NB:(ib + 1) * NB],
                                start=(hd == hd0), stop=(hd == 7))
                        if n < SPLIT_N:
                            nc.vector.tensor_add(
                                osb[:, ib * NB:(ib + 1) * NB], fps[:],
                                osba[(n, ib)][:])
                        else:
                            nc.scalar.copy(
                                osb[:, ib * NB:(ib + 1) * NB], fps[:])
                        nc.sync.dma_start(
                            outA[n * P:(n + 1) * P,
                                 ib * NB:(ib + 1) * NB],
                            osb[:, ib * NB:(ib + 1) * NB])

    nc.compile()
    return nc


def host_inputs(x, freqs_cos, freqs_sin, wq, wk, wv, wo):
    """Build the 8 per-core input maps."""
    bf16 = ml_dtypes.bfloat16
    x = np.asarray(x, np.float32)
    cos = np.asarray(freqs_cos, np.float32)
    sin = np.asarray(freqs_sin, np.float32)
    wq = np.asarray(wq, np.float32)
    wk = np.asarray(wk, np.float32)
    wv = np.asarray(wv, np.float32)
    wo = np.asarray(wo, np.float32)

    perm = np.concatenate([np.arange(0, D, 2), np.arange(1, D, 2)])

    # cos/sin tiles in de-interleaved layout, [128, S] (two 64-row heads)
    cc = cos.T  # [32, S]
    ss = sin.T
    cos64 = np.concatenate([cc, cc], 0)
    # PRE-swapped sin: the kernel computes u = q * sin_swapped then
    # partition-swaps u, so sin rows are stored already swapped (+s, -s)
    sin64 = np.concatenate([ss, -ss], 0)
    cosP = np.ascontiguousarray(np.concatenate([cos64, cos64], 0))
    sinP = np.ascontiguousarray(np.concatenate([sin64, sin64], 0))

    # 128x128 lower-triangle mask for the diagonal key chunks
    j = np.arange(P)[:, None]
    i = np.arange(P)[None, :]
    maskb = np.ascontiguousarray((j <= i).astype(bf16))

    scale = np.float32(1.0 / np.sqrt(D))
    in_maps = []
    for core in range(8):
        b, g = core // 2, core % 2
        wq_g = wq[:, g * QCOLS:(g + 1) * QCOLS].reshape(DIM, HL, D)
        wq_g = (wq_g[:, :, perm] * scale).reshape(DIM, QCOLS)
        wk_g = wk[:, g * KCOLS:(g + 1) * KCOLS].reshape(DIM, KVL, D)
        wk_g = wk_g[:, :, perm].reshape(DIM, KCOLS)
        in_maps.append({
            "xT": np.ascontiguousarray(x[b].T).astype(bf16),
            "wq": np.ascontiguousarray(wq_g).astype(bf16),
            "wk": np.ascontiguousarray(wk_g).astype(bf16),
            "wv": np.ascontiguousarray(
                wv[:, g * KCOLS:(g + 1) * KCOLS]).astype(bf16),
            "wo": np.ascontiguousarray(
                wo[g * QCOLS:(g + 1) * QCOLS, :]).astype(bf16),
            "cosP": cosP.astype(bf16),
            "sinP": sinP.astype(bf16),
            "maskb": maskb,
        })
    return in_maps


_PROGRAM = None


def kernel(x, freqs_cos, freqs_sin, wq, wk, wv, wo):
    global _PROGRAM
    if _PROGRAM is None:
        _PROGRAM = build_program()
    nc = _PROGRAM
    in_maps = host_inputs(x, freqs_cos, freqs_sin, wq, wk, wv, wo)
    trace = os.environ.get("KERNEL_TRACE", "") == "1"
    if not trace:
        # the axon build here lacks the NTFF profile hook; make sure an
        # ambient BASS_TRACE can't route us into that (crashing) path
        os.environ["BASS_NEVER_TRACE"] = "1"
    res = run_bass_kernel_spmd(nc, in_maps, core_ids=list(range(8)),
                               trace=trace)
    if trace and res.exec_time_ns is not None:
        print(f"HW exec time: {res.exec_time_ns} ns")
        print(f"mean exec time: {res.mean_exec_time_ns} ns")
        if res.instructions_and_trace is not None:
            print("trace:", res.instructions_and_trace[1])
    out = np.zeros((B, S, DIM), np.float32)
    for core in range(8):
        b = core // 2
        out[b] += np.asarray(res.results[core]["outA"]).astype(np.float32).T
    return out
